# revision 31
# baseline (speedup 1.0000x reference)
"""NormalizedMutualInformationLoss Trainium2 kernel (v2).

Strategy (data-parallel over batch, 8 batches/core on 8 cores):
  - Device loads only the even rows of x/y via strided DMA; even-column
    selection happens in on-chip reads (strided APs are free for ACT/GP
    and for the one DVE convert op).
  - The reference's noise term (1e-4 * randn) is dropped: it moves the
    final scalar by ~6e-6 relative, far below the 2e-2 gate.
  - Binning: bin i  <=>  th_i <= a < th_{i+1} with a = 12*x, th = m - 12.
    Per threshold m in 1..23 an indicator slab E[m] = step(a - th_m) is
    built over both tensors (x cols 0:512, y cols 512:1024 of each slab).
    Slab 0 is constant ones (written once at startup).
  - Engine split per batch (balanced against the cost model; each engine
    lands at ~5.0-5.3us/batch):
      DVE    : 16 central thresholds (m=4..19) as f16 tensor_scalar in 4x
               perf mode (packed f16 in/out, 327ns/slab) + 64 cols of the
               split threshold.
      ACT    : 4 tail thresholds {1,2,3,20} + the x-half of threshold 21
               via Sign(12*x + bias) on the raw f32 (sign-encoded slabs;
               recovered linearly on the host), plus the PSUM evacs.
      GPSIMD : the f32->f16 convert of a, thresholds {22,23}, and most of
               threshold 21's y-half (is_ge on strided raw f32).
  - Pair-CDF counts S[m,l] = sum_n E_x[n,m] * E_y[n,l] via 512 per-chunk
    PE matmuls [24x24] (contraction 128) accumulated in one PSUM tile per
    batch; LoadStationary is free, so only 24 moving cols per chunk cost
    (5.12us/batch at the full-speed 0.4167 ns/row).
  - Edge batches (0, 6, 7) are emitted at row-half granularity so the
    pipeline fills early and drains fast; batch 7 accumulates in two PSUM
    chains so the final drain is one half-batch.  Evac of batch b is
    deferred into batch b+1's emission so it never head-of-line-blocks
    the ACT queue; each batch's S block is DMA'd out as soon as ready.
  - Host: undo the sign encoding (linear), J = 2D first difference of S,
    then the tiny NMI log-math tail in fp32 (mirrors the reference).
"""

import numpy as np

NB = 24            # histogram bins
B = 64             # total batch
NCORES = 8
BPC = B // NCORES  # batches per core
H = W = 512
F = 512            # data cols per tensor per batch (2 rows x 256 even cols)
SLAB = 2 * F       # slab width: x cols then y cols
NM = NB            # slabs: m=0 ones + m=1..23 thresholds

DVE_SET = set(range(4, 20))   # 16 full slabs on DVE (f16, 4x mode)
ACT_SET = {1, 2, 3, 20}       # full slabs on ACT via Sign (host-recovered)
GP_SET = {22, 23}             # full slabs on GPSIMD (+ the f32->f16 convert)
SPLIT_M = 21                  # x-half on ACT (Sign), y-half on GPSIMD
SPLIT_DVE = 64                # trailing y-half cols of SPLIT_M done on DVE
DVE_EDGE_SPLIT = 16           # DVE slabs emitted at half width on edge batches
ROW_SGN = sorted(ACT_SET | {SPLIT_M})   # sign-encoded x-side slabs
COL_SGN = sorted(ACT_SET)               # sign-encoded y-side slabs

_CACHE = {}


def _split_multi_waits(nc, mybir):
    """This container's walrus accepts only one sync-wait per instruction;
    split multi-wait instructions into single-wait NoOps + the original."""
    cnt = 0
    for fn in nc.m.functions:
        for blk in fn.blocks:
            new = []
            changed = False
            for ins in blk.instructions:
                si = ins.sync_info
                if si is not None and si.on_wait and len(si.on_wait) > 1:
                    waits = list(si.on_wait)
                    for k, w in enumerate(waits[:-1]):
                        nop = mybir.InstNoOp(name=f"{ins.name}_wsplit{k}")
                        nop.engine = ins.engine
                        nop.sync_info = type(si)(on_wait=[w], on_update=[])
                        new.append(nop)
                        cnt += 1
                    ins.sync_info = type(si)(on_wait=[waits[-1]],
                                             on_update=list(si.on_update))
                    changed = True
                new.append(ins)
            if changed:
                blk.instructions = new
    return cnt


def _build_nc():
    import concourse.bass as bass
    import concourse.mybir as mybir
    import concourse.tile as tile

    f32 = mybir.dt.float32
    f16 = mybir.dt.float16
    ige = mybir.AluOpType.is_ge
    Sign = mybir.ActivationFunctionType.Sign

    nc = bass.Bass(trn_type="TRN2")
    xin = nc.dram_tensor("x", [BPC, H, W], f32, kind="ExternalInput")
    yin = nc.dram_tensor("y", [BPC, H, W], f32, kind="ExternalInput")
    sout = nc.dram_tensor("s", [NB, (BPC + 1) * NB], f32,
                          kind="ExternalOutput")

    # Sign bias values for the ScalarE ops; materialized as an in-context
    # tile below (no pre-context barrier needed that way)
    act_bias = {m: float(-(m - 12)) for m in sorted(ACT_SET | {SPLIT_M})}
    bias_cols = sorted(set(act_bias.values()) | {0.0})

    with tile.TileContext(nc) as tc:
        with (
            tc.tile_pool(name="raw", bufs=3) as raw_pool,
            tc.tile_pool(name="aw", bufs=2) as a_pool,
            tc.tile_pool(name="exp", bufs=1) as exp_pool,
            tc.tile_pool(name="psum", bufs=3, space="PSUM") as psum_pool,
            tc.tile_pool(name="sacc", bufs=1) as s_pool,
        ):
            s_all = s_pool.tile([NB, (BPC + 1) * NB], f32)

            # Sign bias columns as a tracked tile (one memset per value)
            cbias = s_pool.tile([128, len(bias_cols)], f32)
            for k, v in enumerate(bias_cols):
                nc.vector.memset(cbias[:, k:k + 1], v)
            bias_ap = {v: cbias[:, k:k + 1] for k, v in enumerate(bias_cols)}

            e_tiles = []
            for i in range(2):
                e_buf = exp_pool.tile([128, NM * SLAB], f16, tag=f"e{i}")
                e_tiles.append(e_buf)
            for et in e_tiles:
                nc.vector.memset(et[:, 0:SLAB], 1.0)  # ones slab, written once
            # warm the ScalarE Sign table during the initial DMA wait
            nc.scalar.activation(e_tiles[0][:, 0:1], e_tiles[0][:, 0:1],
                                 Sign, bias=bias_ap[0.0], scale=1.0)

            def emit_vector(ev, av, rv, jj, conv_split_t=False,
                            dve_split=None, other_split=True):
                """conv + all threshold slabs for column half jj (or full).

                dve_split: when jj is a half, only the first `dve_split` DVE
                slabs are emitted at half width; the rest are emitted at FULL
                width during the j=0 pass (and skipped on the j=1 pass) —
                trims the per-op init premium of edge-batch splitting.
                """
                half_pass = jj != slice(None) and jj.start is not None
                j_idx = jj.start if half_pass else None
                full = slice(None)
                # conv unblocks all DVE ops: pin it ahead of same-engine work
                # (offset must clear a whole batch's emission span, ~600)
                if conv_split_t:
                    if j_idx in (None, 0):
                        with tc.high_priority(offset=600):
                            j0 = slice(0, 1)
                            nc.gpsimd.tensor_copy(av[:, 0:1, j0],
                                                  rv[:, 0:1, j0])
                            nc.gpsimd.tensor_copy(av[:, 1:2, j0],
                                                  rv[:, 1:2, j0])
                        j1 = slice(1, 2)
                        nc.gpsimd.tensor_copy(av[:, 0:1, j1], rv[:, 0:1, j1])
                        nc.gpsimd.tensor_copy(av[:, 1:2, j1], rv[:, 1:2, j1])
                elif dve_split is not None and half_pass:
                    if j_idx == 0:  # full conv feeds the full-width DVE tail
                        with tc.high_priority(offset=600):
                            nc.gpsimd.tensor_copy(av[:, :, :], rv[:, :, :])
                else:
                    with tc.high_priority(offset=600):
                        nc.gpsimd.tensor_copy(av[:, :, jj], rv[:, :, jj])

                dve_seen = 0
                for m in range(1, NB):
                    th_x = float((m - 12) / 12.0)
                    if m in DVE_SET:
                        dve_seen += 1
                        if (dve_split is not None and half_pass
                                and dve_seen > dve_split):
                            if j_idx == 0:  # full-width op on the j0 pass
                                nc.vector.tensor_scalar(
                                    ev[:, m, :, :], av[:, :, :],
                                    th_x, None, ige)
                            # j1 pass: already covered
                        else:
                            nc.vector.tensor_scalar(
                                ev[:, m, :, jj], av[:, :, jj],
                                th_x, None, ige)
                        continue
                    # ACT / GP / SPLIT_M slabs: optionally full-width on the
                    # j0 pass only (their gating rarely drives the schedule)
                    sl = jj
                    if not other_split and half_pass:
                        if j_idx != 0:
                            continue
                        sl = full
                    if m in ACT_SET:
                        nc.scalar.activation(
                            ev[:, m, :, sl], rv[:, :, sl], Sign,
                            bias=bias_ap[act_bias[m]], scale=12.0)
                    elif m in GP_SET:
                        nc.gpsimd.tensor_scalar(
                            ev[:, m, :, sl], rv[:, :, sl], th_x, None, ige)
                    else:  # SPLIT_M: x-half ACT; y-half GP except the
                        # trailing SPLIT_DVE cols on DVE for fine balance
                        nc.scalar.activation(
                            ev[:, m, 0, sl], rv[:, 0, sl], Sign,
                            bias=bias_ap[act_bias[m]], scale=12.0)
                        yv = ev[:, m, 1, sl].rearrange("p j c -> p (j c)")
                        ry = rv[:, 1, sl].rearrange("p j c -> p (j c)")
                        n = yv.shape[1]
                        cut = n - SPLIT_DVE
                        nc.gpsimd.tensor_scalar(yv[:, 0:cut], ry[:, 0:cut],
                                                th_x, None, ige)
                        nc.vector.tensor_scalar(yv[:, cut:n],
                                                av[:, 1, sl].rearrange(
                                                    "p j c -> p (j c)")[:, cut:n],
                                                th_x, None, ige)

            def emit_evac(psum_t, slot, dma=True):
                nc.scalar.activation(
                    s_all[0:NB, slot * NB:(slot + 1) * NB], psum_t[:, :],
                    mybir.ActivationFunctionType.Copy, bias=0.0, scale=1.0)
                if dma:
                    nc.sync.dma_start(sout[:, slot * NB:(slot + 1) * NB],
                                      s_all[0:NB, slot * NB:(slot + 1) * NB])

            pending = None  # (psum_tile, slot) awaiting evac
            for b in range(BPC):
                last = b == BPC - 1
                split = b in (0, BPC - 2, BPC - 1)
                # raw even rows of both tensors: [128, t, j, 512] f32,
                # partition p holds downsampled rows (2p, 2p+1); the j-th
                # sub-row maps to data-column block [j*256, (j+1)*256)
                raw = raw_pool.tile([128, 2, 2, W], f32, tag="raw")
                for j in ((0, 1) if b == 0 else (None,)):
                    for t, src in ((0, xin), (1, yin)):
                        if j is None:
                            nc.sync.dma_start(
                                raw[:, t],
                                src[b, 0:H:2, :].rearrange(
                                    "(p j) c -> p j c", j=2))
                        else:
                            nc.sync.dma_start(raw[:, t, j],
                                              src[b, 2 * j:H:4, :])

                # even-column views of raw (f32, strided)
                rv = raw[:].rearrange("p t j (c two) -> p t j c two",
                                      two=2)[:, :, :, :, 0]
                a = a_pool.tile([128, SLAB], f16, tag="a")
                av = a[:].rearrange("p (t j c) -> p t j c", t=2, j=2)
                e = e_tiles[b % 2]
                ev = e[:].rearrange("p (m t j c) -> p m t j c",
                                    m=NM, t=2, j=2)
                exc = e[:].rearrange("p (m s) -> p s m", m=NM)

                def emit_matmuls(psum_t, c0, c1, start, stop):
                    for c in range(c0, c1):
                        nc.tensor.matmul(
                            psum_t[:, :], exc[:, c, :], exc[:, F + c, :],
                            start=(start and c == c0),
                            stop=(stop and c == c1 - 1))

                if not last:
                    halves = ((0, 1) if split else (None,))
                    for j in halves:
                        jj = slice(None) if j is None else slice(j, j + 1)
                        emit_vector(ev, av, rv, jj,
                                    conv_split_t=(b == 0),
                                    dve_split=DVE_EDGE_SPLIT,
                                    other_split=True)
                    # deferred evac of the previous batch: after this batch's
                    # vector ops so it never head-of-line-blocks ACT
                    if pending is not None:
                        emit_evac(*pending)
                        pending = None
                    psum = psum_pool.tile([NB, NB], mybir.dt.float32, tag="ps")
                    for j in halves:
                        if j is None:
                            emit_matmuls(psum, 0, F, True, True)
                        else:
                            emit_matmuls(psum, j * (F // 2),
                                         (j + 1) * (F // 2), j == 0, j == 1)
                    pending = (psum, b)
                else:
                    # last batch: two PSUM chains so the drain is short
                    emit_vector(ev, av, rv, slice(0, 1),
                                dve_split=DVE_EDGE_SPLIT, other_split=True)
                    if pending is not None:
                        emit_evac(*pending)
                        pending = None
                    psum_a = psum_pool.tile([NB, NB], mybir.dt.float32,
                                            tag="ps")
                    emit_matmuls(psum_a, 0, F // 2, True, True)
                    emit_vector(ev, av, rv, slice(1, 2),
                                dve_split=DVE_EDGE_SPLIT, other_split=True)
                    emit_evac(psum_a, b)
                    psum_b = psum_pool.tile([NB, NB], mybir.dt.float32,
                                            tag="ps")
                    emit_matmuls(psum_b, F // 2, F, True, True)
                    emit_evac(psum_b, BPC)

    _split_multi_waits(nc, mybir)
    return nc


def _get_nc():
    if "nc" not in _CACHE:
        _CACHE["nc"] = _build_nc()
    return _CACHE["nc"]


def _recover_steps(R):
    """Undo the ACT Sign encoding. ROW_SGN x-side / COL_SGN y-side slabs
    hold sign(a - th) = 2*step - 1; all other slabs hold clean steps, and
    row/col 0 are exact ones. Linear, exact in f64."""
    S = R.copy()
    # rows first: R[m, :] = 2*S1[m, :] - R[0, :]  (row 0 = sum over all n)
    S[:, ROW_SGN, :] = (R[:, ROW_SGN, :] + R[:, 0:1, :]) / 2.0
    # then cols, using the row-fixed matrix
    S[:, :, COL_SGN] = (S[:, :, COL_SGN] + S[:, :, 0:1]) / 2.0
    return S


def _nmi_tail(S):
    """S: [B, 24, 24] pair-CDF counts (S[m,l] = #{ax>=th_m & ay>=th_l},
    th_0 = -inf).  J = 2D first difference; then the reference's fp32
    NMI math."""
    Se = np.zeros((S.shape[0], NB + 1, NB + 1), np.float64)
    Se[:, 0:NB, 0:NB] = S
    J = (Se[:, 0:NB, 0:NB] - Se[:, 1:NB + 1, 0:NB]
         - Se[:, 0:NB, 1:NB + 1] + Se[:, 1:NB + 1, 1:NB + 1]).astype(np.float32)
    total = J.sum(axis=(1, 2), keepdims=True).astype(np.float32) + np.float32(1e-10)
    joint = (J / total).astype(np.float32)
    x_hist = joint.sum(axis=2, dtype=np.float32)
    y_hist = joint.sum(axis=1, dtype=np.float32)
    eps = np.float32(1e-5)
    joint_e = joint + eps
    xh = x_hist + eps
    yh = y_hist + eps
    log_joint = np.log(joint_e)
    log_prod = np.log(xh[:, :, None] * yh[:, None, :])
    mi = np.sum(joint_e * (log_joint - log_prod), axis=(1, 2), dtype=np.float32)
    hx = -np.sum(xh * np.log(xh), axis=1, dtype=np.float32)
    hy = -np.sum(yh * np.log(yh), axis=1, dtype=np.float32)
    se = hx + hy
    nmi = np.where(se < np.float32(1e-10), np.float32(0.0),
                   np.float32(2.0) * mi / se)
    nmi = np.clip(nmi, -1.0, 1.0).astype(np.float32)
    return np.float32(-np.clip(np.mean(nmi, dtype=np.float32), -1.0, 1.0))


def _run_device(x, y, trace=False):
    from concourse.bass_utils import run_bass_kernel_spmd
    nc = _get_nc()
    x = np.ascontiguousarray(np.asarray(x).reshape(B, H, W), dtype=np.float32)
    y = np.ascontiguousarray(np.asarray(y).reshape(B, H, W), dtype=np.float32)
    in_maps = [
        {"x": x[c * BPC:(c + 1) * BPC], "y": y[c * BPC:(c + 1) * BPC]}
        for c in range(NCORES)
    ]
    res = run_bass_kernel_spmd(nc, in_maps, core_ids=list(range(NCORES)),
                               trace=trace)
    S = np.zeros((B, NB, NB), dtype=np.float64)
    for c in range(NCORES):
        sc = res.results[c]["s"].astype(np.float64)
        for b in range(BPC):
            S[c * BPC + b] = sc[:, b * NB:(b + 1) * NB]
        # last batch was accumulated in two PSUM chains (slots BPC-1, BPC)
        S[c * BPC + BPC - 1] += sc[:, BPC * NB:(BPC + 1) * NB]
    S = _recover_steps(S)
    return S, res


def kernel(x, y):
    S, _ = _run_device(x, y)
    return _nmi_tail(S)
